# revision 15
# baseline (speedup 1.0000x reference)
"""Trainium2 Bass kernel for nn_Blocks2Matrix (scatter_memory).

Strategy (all index math is resolved at trace time, so the scatter becomes a
fully static schedule):
 - Shard systems across the 8 cores (2 systems/core); bucket pair entries by
   (system, row atom). Each pair contributes a direct entry (row=i, col=j)
   and a transposed entry (row=j, col=i, V^T).
 - Device layout ("K-layout"): for each system a [2560, 2560] matrix indexed
   [(i, p, a), (j, q, b)] is built as 25 planes [512, 512] indexed (a, b);
   every entry contribution is then matmul-native.
 - Scatter stage: per slab (sys_local, row atom i) and entry type, chunks of
   16 entries are accumulated into PSUM via one-hot matmuls:
       acc[(p*5+mu), c] += VA_chunk[128, 40].T @ onehot[128, 512]
   where onehot rows are (entry, q) and the single 1.0 sits at col j*8+q.
 - Dense stage: one matmul per (slab, pg in {0,1}):
       out[(pp, a, b) = 100, c] = BD[80, 100].T @ acc80[80, 512]
   with BD a block-diagonal of cg (direct half) and cg^T (transposed half).
 - Output DMA'd straight from PSUM to DRAM; host permutes to the reference
   layout and concatenates the cores' systems.
"""
import contextlib

import numpy as np

import concourse.bass as bass
import concourse.mybir as mybir
from concourse.tile import TileContext
from concourse.bass_utils import run_bass_kernel_spmd

N_SYS, N_ATOMS, NRAD, MU, M1, M2 = 16, 64, 8, 5, 5, 5
S = 32768
NORB = NRAD * M1            # 40
N = N_ATOMS * NORB          # 2560
N_CORES = 8
SYS_PER_CORE = N_SYS // N_CORES
NK = N_ATOMS * NRAD         # 512
CHUNK = 16                  # entries per scatter matmul (K = 128)
N_SLABS = SYS_PER_CORE * N_ATOMS   # 128 per core
TRA_BASE = 64               # partition base of the transposed slab half
VA_TILE_CHUNKS = 50         # chunks per SBUF va tile -> [128, 2000] = 1 MB
F32 = mybir.dt.float32


def _preprocess(values, sys_idx, i_idx, j_idx):
    """Build per-core SBUF images. Returns (va_img, jcol, Ck).

    va_img [8, 128, nchunk*40]  : chunk k's lhsT is cols [k*40, (k+1)*40)
    jcol   [8, 128, nchunk] f32 : chunk k's one-hot target col per (e, q) row
    Ck     [N_SLABS, 2]         : chunks per (slab, entry type), shared by cores
    """
    vals = values.reshape(S, MU, NRAD, NRAD)
    sys_idx = np.asarray(sys_idx, dtype=np.int64)
    i_idx = np.asarray(i_idx, dtype=np.int64)
    j_idx = np.asarray(j_idx, dtype=np.int64)

    ent_sys = np.concatenate([sys_idx, sys_idx])
    ent_row = np.concatenate([i_idx, j_idx])
    ent_col = np.concatenate([j_idx, i_idx])
    ent_typ = np.concatenate([np.zeros(S, np.int64), np.ones(S, np.int64)])

    core_of = ent_sys // SYS_PER_CORE
    slab_of = (ent_sys % SYS_PER_CORE) * N_ATOMS + ent_row

    flat = (core_of * N_SLABS + slab_of) * 2 + ent_typ
    counts = np.bincount(flat, minlength=N_CORES * N_SLABS * 2) \
               .reshape(N_CORES, N_SLABS, 2)
    Ck = np.maximum(1, -(-counts // CHUNK)).max(axis=0)      # [N_SLABS, 2]
    E_grp = (Ck * CHUNK).reshape(-1)
    E_tot = int(E_grp.sum())
    grp_base = np.concatenate([[0], np.cumsum(E_grp)[:-1]])

    order = np.lexsort((ent_col, ent_typ, ent_row, ent_sys))
    oc = core_of[order]
    ogrp = slab_of[order] * 2 + ent_typ[order]
    key = oc * (N_SLABS * 2) + ogrp
    first = np.r_[True, key[1:] != key[:-1]]
    idx = np.arange(key.size)
    start_of_group = np.maximum.accumulate(np.where(first, idx, 0))
    rank = idx - start_of_group
    dest = grp_base[ogrp] + rank

    # VA[e, q, p*5+mu] = V[mu, p, q]; transposed entries use V^T in (p, q)
    VA_dir = vals.transpose(0, 3, 2, 1).reshape(S, NRAD, NORB)
    VA_tra = vals.transpose(0, 2, 3, 1).reshape(S, NRAD, NORB)
    ent_VA = np.concatenate([VA_dir, VA_tra], axis=0)

    va = np.zeros((N_CORES, E_tot, NRAD, NORB), dtype=np.float32)
    va[oc, dest] = ent_VA[order]
    jq = np.zeros((N_CORES, E_tot, NRAD), dtype=np.float32)
    jq[oc, dest] = (ent_col[order, None] * NRAD + np.arange(NRAD)[None, :]).astype(np.float32)

    va_flat = va.reshape(N_CORES, E_tot * NRAD, NORB)
    nchunk = E_tot * NRAD // 128
    va_img = va_flat.reshape(N_CORES, nchunk, 128, NORB).transpose(0, 2, 1, 3) \
                    .reshape(N_CORES, 128, nchunk * NORB).copy()
    jcol = jq.reshape(N_CORES, nchunk, 128).transpose(0, 2, 1).copy()
    return va_img, jcol, Ck


def _make_bd(cg):
    """BD [104, 200]: rows 0:40 direct cg, rows 64:104 transposed cg (rows
    40:64 zero -- the transposed slab half sits at partition 64 so that all
    engine accesses start at 32-aligned partitions); cols pg*100."""
    cgm_dir = cg.reshape(M1 * M2, MU).T.astype(np.float32)                    # [mu, a*5+b]
    cgm_tra = np.ascontiguousarray(cg.transpose(1, 0, 2)).reshape(M1 * M2, MU).T.astype(np.float32)
    bd = np.zeros((TRA_BASE + NORB, 2 * 100), dtype=np.float32)
    for pg in range(2):
        for pp in range(4):
            p = pg * 4 + pp
            bd[p * 5:(p + 1) * 5, pg * 100 + pp * 25: pg * 100 + (pp + 1) * 25] = cgm_dir
            bd[TRA_BASE + p * 5:TRA_BASE + (p + 1) * 5, pg * 100 + pp * 25: pg * 100 + (pp + 1) * 25] = cgm_tra
    return bd


def _postprocess(outs):
    """outs: [8][N_SLABS*2*100, NK] -> H [N_SYS, N, N]."""
    K = np.stack(outs).reshape(N_CORES * SYS_PER_CORE, N_ATOMS, 2, 4, M1, M2, N_ATOMS, NRAD)
    return np.ascontiguousarray(
        K.transpose(0, 1, 2, 3, 4, 6, 7, 5)
    ).reshape(N_SYS, N, N)


def _build_program(Ck, nchunk):
    """Raw-bass SPMD program (explicit semaphores; no Tile).

    Engine pipeline per chunk k / group g = slab*2+type / slab s:
      DVE : one-hot[k%8] = is_equal(iota, jc[:, k])            -> oh_sem
      PE  : acc[g%4] += va[k].T @ oh[k%8]   (PSUM accumulate)  -> mm_sem
      ACT : slab_sb[s%4] rows {0,64}+40 <- acc copies          -> cp_sem
      PE  : pout[s%2][pg] = BD[pg].T @ slab_sb[s%4]            -> ds_sem
      ACT : stage[s%4] <- pout[s%2]                            -> stg_sem
      SYNC: DRAM out rows [s*200, (s+1)*200) <- stage[s%4]     -> out_sems[s%4]
    """
    nc = bass.Bass()
    W = nchunk * NORB
    n_va_tiles = -(-nchunk // VA_TILE_CHUNKS)

    va_d = nc.declare_dram_parameter("va", [128, W], F32, isOutput=False)
    jc_d = nc.declare_dram_parameter("jcol", [128, nchunk], F32, isOutput=False)
    bd_d = nc.declare_dram_parameter("bd", [TRA_BASE + NORB, 200], F32, isOutput=False)
    out_d = nc.declare_dram_parameter("out", [N_SLABS * 2 * 100, NK], F32, isOutput=True)

    # static schedule
    groups = []           # (n_chunks, first_k)
    k0 = 0
    for slab in range(N_SLABS):
        for t in range(2):
            nck = int(Ck[slab, t])
            groups.append((nck, k0))
            k0 += nck
    cum_mm = np.cumsum([g[0] for g in groups])        # mm_sem value after group g

    with (
        nc.sbuf_tensor([128, W], F32) as va_sb,
        nc.sbuf_tensor([128, nchunk], F32) as jc_sb,
        nc.sbuf_tensor([TRA_BASE + NORB, 200], F32) as bd_sb,
        nc.sbuf_tensor([128, NK], F32) as iota_sb,
        nc.sbuf_tensor([128, 8 * NK], F32) as oh_sb,
        nc.sbuf_tensor([TRA_BASE + NORB, 4 * NK], F32) as slab_sb,
        nc.sbuf_tensor([100, 4 * 2 * NK], F32) as stage_sb,
        nc.psum_tensor([NORB, 4 * NK], F32) as acc_ps,
        nc.psum_tensor([100, 2 * 2 * NK], F32) as pout_ps,
        nc.semaphore("cst_sem") as cst_sem,
        nc.semaphore("oh_sem") as oh_sem,
        nc.semaphore("mm_sem") as mm_sem,
        nc.semaphore("ds_sem") as ds_sem,
        nc.semaphore("cp_sem") as cp_sem,
        nc.semaphore("stg_sem") as stg_sem,
        nc.Block() as block,
    ):
        with contextlib.ExitStack() as stk:
            va_sems = [stk.enter_context(nc.semaphore(f"va_sem{t}"))
                       for t in range(n_va_tiles)]
            out_sems = [stk.enter_context(nc.semaphore(f"out_sem{i}"))
                        for i in range(4)]

            def oh_slice(k):
                return oh_sb[:, (k % 8) * NK:(k % 8 + 1) * NK]

            def acc_slice(g):
                return acc_ps[:, (g % 4) * NK:(g % 4 + 1) * NK]

            def slab_slice(s):
                return slab_sb[:, (s % 4) * NK:(s % 4 + 1) * NK]

            def pout_slice(s, pg):
                return pout_ps[:, ((s % 2) * 2 + pg) * NK:((s % 2) * 2 + pg + 1) * NK]

            def stage_slice(s):
                return stage_sb[:, (s % 4) * 2 * NK:((s % 4) + 1) * 2 * NK]

            @block.gpsimd
            def _(gpsimd):
                nc.gpsimd.iota(iota_sb[:], pattern=[[1, NK]], base=0,
                               channel_multiplier=0,
                               allow_small_or_imprecise_dtypes=True).then_inc(cst_sem, 1)

            @block.sync
            def _(sync):
                for t in range(n_va_tiles):
                    c0 = t * VA_TILE_CHUNKS * NORB
                    c1 = min(W, (t + 1) * VA_TILE_CHUNKS * NORB)
                    sync.dma_start(out=va_sb[:, c0:c1],
                                   in_=va_d[:, c0:c1]).then_inc(va_sems[t], 16)
                sync.dma_start(out=jc_sb[:], in_=jc_d[:]).then_inc(cst_sem, 16)
                sync.dma_start(out=bd_sb[:], in_=bd_d[:]).then_inc(cst_sem, 16)
                for s in range(N_SLABS):
                    sync.wait_ge(stg_sem, s + 1)
                    dst = out_d[s * 200:(s + 1) * 200, :] \
                        .rearrange("(pg r) c -> r pg c", pg=2)
                    sync.dma_start(
                        out=dst,
                        in_=stage_slice(s).rearrange("r (pg c) -> r pg c", pg=2),
                    ).then_inc(out_sems[s % 4], 16)

            @block.vector
            def _(vector):
                vector.wait_ge(cst_sem, 33)
                for k in range(k0):
                    if k >= 8:
                        vector.wait_ge(mm_sem, k - 7)
                    nc.vector.tensor_scalar(
                        out=oh_slice(k), in0=iota_sb[:],
                        scalar1=jc_sb[:, k:k + 1], scalar2=None,
                        op0=mybir.AluOpType.is_equal).then_inc(oh_sem, 1)

            @block.tensor
            def _(tensor):
                tensor.wait_ge(cst_sem, 33)

                def dense(s):
                    tensor.wait_ge(cp_sem, 2 * (s + 1))
                    if s >= 2:
                        tensor.wait_ge(stg_sem, s - 1)
                    for pg in range(2):
                        nc.tensor.matmul(
                            pout_slice(s, pg),
                            bd_sb[:, pg * 100:(pg + 1) * 100],
                            slab_sb[:, (s % 4) * NK:(s % 4 + 1) * NK],
                            start=True, stop=True).then_inc(ds_sem, 1)

                cur_tile = -1
                k = 0
                for s in range(N_SLABS):
                    for t in range(2):
                        g = s * 2 + t
                        nck = groups[g][0]
                        for kk in range(nck):
                            tt = k // VA_TILE_CHUNKS
                            if tt != cur_tile:
                                tensor.wait_ge(va_sems[tt], 16)
                                cur_tile = tt
                            tensor.wait_ge(oh_sem, k + 1)
                            if kk == 0 and g >= 4:
                                tensor.wait_ge(cp_sem, g - 3)
                            nc.tensor.matmul(
                                acc_slice(g),
                                va_sb[:, k * NORB:(k + 1) * NORB],
                                oh_slice(k),
                                start=(kk == 0), stop=(kk == nck - 1)).then_inc(mm_sem, 1)
                            k += 1
                    if s >= 1:
                        dense(s - 1)
                dense(N_SLABS - 1)

            @block.scalar
            def _(scalar):
                scalar.wait_ge(cst_sem, 33)
                # zero the 40:64 partition gap of each slab slot once
                # (32:64 is the closest 32-aligned base; copies overwrite 32:40)
                for sl in range(4):
                    nc.scalar.activation(
                        out=slab_sb[32:TRA_BASE, sl * NK:(sl + 1) * NK],
                        in_=iota_sb[32:TRA_BASE, :],
                        func=mybir.ActivationFunctionType.Copy, scale=0.0)

                def stage(s):
                    scalar.wait_ge(ds_sem, 2 * (s + 1))
                    if s >= 4:
                        scalar.wait_ge(out_sems[s % 4], 16 * (s // 4))
                    nc.scalar.copy(
                        out=stage_slice(s),
                        in_=pout_ps[:, (s % 2) * 2 * NK:((s % 2) + 1) * 2 * NK],
                    ).then_inc(stg_sem, 1)

                for s in range(N_SLABS):
                    for t in range(2):
                        g = s * 2 + t
                        scalar.wait_ge(mm_sem, int(cum_mm[g]))
                        if s >= 4 and t == 0:
                            scalar.wait_ge(ds_sem, 2 * (s - 3))
                        nc.scalar.copy(
                            out=slab_sb[t * TRA_BASE:t * TRA_BASE + NORB,
                                        (s % 4) * NK:(s % 4 + 1) * NK],
                            in_=acc_slice(g)).then_inc(cp_sem, 1)
                    if s >= 1:
                        stage(s - 1)
                stage(N_SLABS - 1)

    return nc


def _run(values, cg, sys_idx, i_idx, j_idx, trace=False):
    va_img, jcol, Ck = _preprocess(values, sys_idx, i_idx, j_idx)
    bd = _make_bd(np.asarray(cg, dtype=np.float32))
    nchunk = int(Ck.sum()) * CHUNK * NRAD // 128
    nc = _build_program(Ck, nchunk)
    in_maps = [{"va": va_img[c], "jcol": jcol[c], "bd": bd}
               for c in range(N_CORES)]
    res = run_bass_kernel_spmd(nc, in_maps, list(range(N_CORES)), trace=trace)
    outs = [res.results[c]["out"] for c in range(N_CORES)]
    return _postprocess(outs), res


def kernel(values, cg, sys_idx, i_idx, j_idx):
    H, _ = _run(np.asarray(values, dtype=np.float32), cg, sys_idx, i_idx, j_idx)
    return H


# revision 16
# speedup vs baseline: 2.8509x; 2.8509x over previous
"""Trainium2 Bass kernel for nn_Blocks2Matrix (scatter_memory).

Strategy (all index math is resolved at trace time, so the scatter becomes a
fully static schedule):
 - Shard systems across the 8 cores (2 systems/core); bucket pair entries by
   (system, row atom). Each pair contributes a direct entry (row=i, col=j)
   and a transposed entry (row=j, col=i, V^T).
 - Device layout ("K-layout"): for each system a [2560, 2560] matrix indexed
   [(i, p, a), (j, q, b)] is built as 25 planes [512, 512] indexed (a, b);
   every entry contribution is then matmul-native.
 - Scatter stage: per slab (sys_local, row atom i) and entry type, chunks of
   16 entries are accumulated into PSUM via one-hot matmuls:
       acc[(p*5+mu), c] += VA_chunk[128, 40].T @ onehot[128, 512]
   where onehot rows are (entry, q) and the single 1.0 sits at col j*8+q.
 - Dense stage: one matmul per (slab, pg in {0,1}):
       out[(pp, a, b) = 100, c] = BD[80, 100].T @ acc80[80, 512]
   with BD a block-diagonal of cg (direct half) and cg^T (transposed half).
 - Output DMA'd straight from PSUM to DRAM; host permutes to the reference
   layout and concatenates the cores' systems.
"""
import contextlib

import numpy as np

import concourse.bass as bass
import concourse.mybir as mybir
from concourse.tile import TileContext
from concourse.bass_utils import run_bass_kernel_spmd

N_SYS, N_ATOMS, NRAD, MU, M1, M2 = 16, 64, 8, 5, 5, 5
S = 32768
NORB = NRAD * M1            # 40
N = N_ATOMS * NORB          # 2560
N_CORES = 8
SYS_PER_CORE = N_SYS // N_CORES
NK = N_ATOMS * NRAD         # 512
CHUNK = 16                  # entries per scatter matmul (K = 128)
N_SLABS = SYS_PER_CORE * N_ATOMS   # 128 per core
TRA_BASE = 64               # partition base of the transposed slab half
VA_TILE_CHUNKS = 50         # chunks per SBUF va tile -> [128, 2000] = 1 MB
F32 = mybir.dt.float32
BF16 = mybir.dt.bfloat16
KDIM = 128              # dense-stage contraction rows (padded for FWL)


def _preprocess(values, sys_idx, i_idx, j_idx):
    """Build per-core SBUF images. Returns (va_img, jcol, Ck).

    va_img [8, 128, nchunk*40]  : chunk k's lhsT is cols [k*40, (k+1)*40)
    jcol   [8, 128, nchunk] f32 : chunk k's one-hot target col per (e, q) row
    Ck     [N_SLABS, 2]         : chunks per (slab, entry type), shared by cores
    """
    vals = values.reshape(S, MU, NRAD, NRAD)
    sys_idx = np.asarray(sys_idx, dtype=np.int64)
    i_idx = np.asarray(i_idx, dtype=np.int64)
    j_idx = np.asarray(j_idx, dtype=np.int64)

    ent_sys = np.concatenate([sys_idx, sys_idx])
    ent_row = np.concatenate([i_idx, j_idx])
    ent_col = np.concatenate([j_idx, i_idx])
    ent_typ = np.concatenate([np.zeros(S, np.int64), np.ones(S, np.int64)])

    core_of = ent_sys // SYS_PER_CORE
    slab_of = (ent_sys % SYS_PER_CORE) * N_ATOMS + ent_row

    flat = (core_of * N_SLABS + slab_of) * 2 + ent_typ
    counts = np.bincount(flat, minlength=N_CORES * N_SLABS * 2) \
               .reshape(N_CORES, N_SLABS, 2)
    Ck = np.maximum(1, -(-counts // CHUNK)).max(axis=0)      # [N_SLABS, 2]
    E_grp = (Ck * CHUNK).reshape(-1)
    E_tot = int(E_grp.sum())
    grp_base = np.concatenate([[0], np.cumsum(E_grp)[:-1]])

    order = np.lexsort((ent_col, ent_typ, ent_row, ent_sys))
    oc = core_of[order]
    ogrp = slab_of[order] * 2 + ent_typ[order]
    key = oc * (N_SLABS * 2) + ogrp
    first = np.r_[True, key[1:] != key[:-1]]
    idx = np.arange(key.size)
    start_of_group = np.maximum.accumulate(np.where(first, idx, 0))
    rank = idx - start_of_group
    dest = grp_base[ogrp] + rank

    # VA[e, q, p*5+mu] = V[mu, p, q]; transposed entries use V^T in (p, q)
    VA_dir = vals.transpose(0, 3, 2, 1).reshape(S, NRAD, NORB)
    VA_tra = vals.transpose(0, 2, 3, 1).reshape(S, NRAD, NORB)
    ent_VA = np.concatenate([VA_dir, VA_tra], axis=0)

    va = np.zeros((N_CORES, E_tot, NRAD, NORB), dtype=np.float32)
    va[oc, dest] = ent_VA[order]
    jq = np.zeros((N_CORES, E_tot, NRAD), dtype=np.float32)
    jq[oc, dest] = (ent_col[order, None] * NRAD + np.arange(NRAD)[None, :]).astype(np.float32)

    va_flat = va.reshape(N_CORES, E_tot * NRAD, NORB)
    nchunk = E_tot * NRAD // 128
    va_img = va_flat.reshape(N_CORES, nchunk, 128, NORB).transpose(0, 2, 1, 3) \
                    .reshape(N_CORES, 128, nchunk * NORB).copy()
    jcol = jq.reshape(N_CORES, nchunk, 128).transpose(0, 2, 1).copy()

    # per-chunk one-hot column spans (union over cores; entries are sorted by
    # col within each group, so spans are narrow). Padding entries (jc=0) are
    # excluded; sliced chunks with c0 > 0 never match them.
    chunk_of = (dest // CHUNK).astype(np.int64)
    cmin = np.full(nchunk, NK, np.int64)
    cmax = np.full(nchunk, -1, np.int64)
    ecol = ent_col[order] * NRAD
    np.minimum.at(cmin, chunk_of, ecol)
    np.maximum.at(cmax, chunk_of, ecol + NRAD)
    empty = cmax < 0
    cmin[empty], cmax[empty] = 0, NRAD
    spans = np.stack([(cmin // 4) * 4, np.minimum(NK, -(-cmax // 4) * 4)], axis=1)
    return va_img, jcol, Ck, spans


def _make_bd(cg):
    """BD [104, 200]: rows 0:40 direct cg, rows 64:104 transposed cg (rows
    40:64 zero -- the transposed slab half sits at partition 64 so that all
    engine accesses start at 32-aligned partitions); cols pg*100."""
    cgm_dir = cg.reshape(M1 * M2, MU).T.astype(np.float32)                    # [mu, a*5+b]
    cgm_tra = np.ascontiguousarray(cg.transpose(1, 0, 2)).reshape(M1 * M2, MU).T.astype(np.float32)
    bd = np.zeros((KDIM, 2 * 100), dtype=np.float32)
    for pg in range(2):
        for pp in range(4):
            p = pg * 4 + pp
            bd[p * 5:(p + 1) * 5, pg * 100 + pp * 25: pg * 100 + (pp + 1) * 25] = cgm_dir
            bd[TRA_BASE + p * 5:TRA_BASE + (p + 1) * 5, pg * 100 + pp * 25: pg * 100 + (pp + 1) * 25] = cgm_tra
    return bd


def _postprocess(outs):
    """outs: [8][N_SLABS*2*100, NK] -> H [N_SYS, N, N]."""
    K = np.stack(outs).reshape(N_CORES * SYS_PER_CORE, N_ATOMS, 2, 4, M1, M2, N_ATOMS, NRAD)
    return np.ascontiguousarray(
        K.transpose(0, 1, 2, 3, 4, 6, 7, 5)
    ).reshape(N_SYS, N, N)


def _build_program(Ck, nchunk, spans):
    """Raw-bass SPMD program (explicit semaphores; no Tile).

    Engine pipeline per chunk k / group g = slab*2+type / slab s:
      DVE : one-hot[k%8] = is_equal(iota, jc[:, k])            -> oh_sem
      PE  : acc[g%4] += va[k].T @ oh[k%8]   (PSUM accumulate)  -> mm_sem
      ACT : slab_sb[s%4] rows {0,64}+40 <- acc copies          -> cp_sem
      PE  : pout[s%2][pg] = BD[pg].T @ slab_sb[s%4]            -> ds_sem
      ACT : stage[s%4] <- pout[s%2]                            -> stg_sem
      SYNC: DRAM out rows [s*200, (s+1)*200) <- stage[s%4]     -> out_sems[s%4]
    """
    nc = bass.Bass()
    W = nchunk * NORB
    n_va_tiles = -(-nchunk // VA_TILE_CHUNKS)

    va_d = nc.declare_dram_parameter("va", [128, W], BF16, isOutput=False)
    jc_d = nc.declare_dram_parameter("jcol", [128, nchunk], F32, isOutput=False)
    bd_d = nc.declare_dram_parameter("bd", [KDIM, 200], BF16, isOutput=False)
    out_d = nc.declare_dram_parameter("out", [N_SLABS * 2 * 100, NK], BF16, isOutput=True)

    # static schedule
    groups = []           # (n_chunks, first_k)
    k0 = 0
    for slab in range(N_SLABS):
        for t in range(2):
            nck = int(Ck[slab, t])
            groups.append((nck, k0))
            k0 += nck
    cum_mm = np.cumsum([g[0] for g in groups])        # mm_sem value after group g
    kspan = spans.copy()
    for nck, first_k in groups:
        kspan[first_k] = (0, NK)

    with (
        nc.sbuf_tensor([128, W], BF16) as va_sb,
        nc.sbuf_tensor([128, nchunk], F32) as jc_sb,
        nc.sbuf_tensor([KDIM, 200], BF16) as bd_sb,
        nc.sbuf_tensor([128, NK], F32) as iota_sb,
        nc.sbuf_tensor([128, 8 * NK], BF16) as oh_sb,
        nc.sbuf_tensor([KDIM, 4 * NK], BF16) as slab_sb,
        nc.sbuf_tensor([100, 4 * 2 * NK], BF16) as stage_sb,
        nc.psum_tensor([NORB, 4 * NK], F32) as acc_ps,
        nc.psum_tensor([100, 2 * 2 * NK], F32) as pout_ps,
        nc.semaphore("cst_sem") as cst_sem,
        nc.semaphore("oh_sem") as oh_sem,
        nc.semaphore("mm_sem") as mm_sem,
        nc.semaphore("ds_sem") as ds_sem,
        nc.semaphore("cp_sem") as cp_sem,
        nc.semaphore("stg_sem") as stg_sem,
        nc.Block() as block,
    ):
        with contextlib.ExitStack() as stk:
            va_sems = [stk.enter_context(nc.semaphore(f"va_sem{t}"))
                       for t in range(n_va_tiles)]
            out_sems = [stk.enter_context(nc.semaphore(f"out_sem{i}"))
                        for i in range(4)]

            def oh_slice(k):
                return oh_sb[:, (k % 8) * NK:(k % 8 + 1) * NK]

            def acc_slice(g):
                return acc_ps[:, (g % 4) * NK:(g % 4 + 1) * NK]

            def slab_slice(s):
                return slab_sb[:, (s % 4) * NK:(s % 4 + 1) * NK]

            def pout_slice(s, pg):
                return pout_ps[:, ((s % 2) * 2 + pg) * NK:((s % 2) * 2 + pg + 1) * NK]

            def stage_slice(s):
                return stage_sb[:, (s % 4) * 2 * NK:((s % 4) + 1) * 2 * NK]

            @block.gpsimd
            def _(gpsimd):
                nc.gpsimd.iota(iota_sb[:], pattern=[[1, NK]], base=0,
                               channel_multiplier=0,
                               allow_small_or_imprecise_dtypes=True).then_inc(cst_sem, 1)

            @block.sync
            def _(sync):
                for t in range(n_va_tiles):
                    c0 = t * VA_TILE_CHUNKS * NORB
                    c1 = min(W, (t + 1) * VA_TILE_CHUNKS * NORB)
                    sync.dma_start(out=va_sb[:, c0:c1],
                                   in_=va_d[:, c0:c1]).then_inc(va_sems[t], 16)
                sync.dma_start(out=jc_sb[:], in_=jc_d[:]).then_inc(cst_sem, 16)
                sync.dma_start(out=bd_sb[:], in_=bd_d[:]).then_inc(cst_sem, 16)
                for s in range(N_SLABS):
                    sync.wait_ge(stg_sem, s + 1)
                    dst = out_d[s * 200:(s + 1) * 200, :] \
                        .rearrange("(pg r) c -> r pg c", pg=2)
                    sync.dma_start(
                        out=dst,
                        in_=stage_slice(s).rearrange("r (pg c) -> r pg c", pg=2),
                    ).then_inc(out_sems[s % 4], 16)

            @block.vector
            def _(vector):
                vector.wait_ge(cst_sem, 33)
                for k in range(k0):
                    c0, c1 = int(kspan[k][0]), int(kspan[k][1])
                    if k >= 8:
                        vector.wait_ge(mm_sem, k - 7)
                    base = (k % 8) * NK
                    nc.vector.tensor_scalar(
                        out=oh_sb[:, base + c0:base + c1],
                        in0=iota_sb[:, c0:c1],
                        scalar1=jc_sb[:, k:k + 1], scalar2=None,
                        op0=mybir.AluOpType.is_equal).then_inc(oh_sem, 1)

            @block.tensor
            def _(tensor):
                tensor.wait_ge(cst_sem, 33)

                def dense(s):
                    tensor.wait_ge(cp_sem, 2 * (s + 1))
                    if s >= 2:
                        tensor.wait_ge(stg_sem, s - 1)
                    for pg in range(2):
                        nc.tensor.matmul(
                            pout_slice(s, pg),
                            bd_sb[:, pg * 100:(pg + 1) * 100],
                            slab_sb[:, (s % 4) * NK:(s % 4 + 1) * NK],
                            start=True, stop=True).then_inc(ds_sem, 1)

                cur_tile = -1
                k = 0
                for s in range(N_SLABS):
                    for t in range(2):
                        g = s * 2 + t
                        nck = groups[g][0]
                        for kk in range(nck):
                            tt = k // VA_TILE_CHUNKS
                            if tt != cur_tile:
                                tensor.wait_ge(va_sems[tt], 16)
                                cur_tile = tt
                            tensor.wait_ge(oh_sem, k + 1)
                            if kk == 0 and g >= 4:
                                tensor.wait_ge(cp_sem, g - 3)
                            c0, c1 = int(kspan[k][0]), int(kspan[k][1])
                            base = (k % 8) * NK
                            ab = (g % 4) * NK
                            nc.tensor.matmul(
                                acc_ps[:, ab + c0:ab + c1],
                                va_sb[:, k * NORB:(k + 1) * NORB],
                                oh_sb[:, base + c0:base + c1],
                                start=(kk == 0), stop=(kk == nck - 1),
                                skip_group_check=True).then_inc(mm_sem, 1)
                            k += 1
                    if s >= 1:
                        dense(s - 1)
                dense(N_SLABS - 1)

            @block.scalar
            def _(scalar):
                scalar.wait_ge(cst_sem, 33)
                # zero the 40:64 partition gap of each slab slot once
                # (32:64 is the closest 32-aligned base; copies overwrite 32:40)
                for sl in range(4):
                    for r0 in (32, 96):
                        nc.scalar.activation(
                            out=slab_sb[r0:r0 + 32, sl * NK:(sl + 1) * NK],
                            in_=iota_sb[r0:r0 + 32, :],
                            func=mybir.ActivationFunctionType.Copy, scale=0.0)

                def stage(s):
                    scalar.wait_ge(ds_sem, 2 * (s + 1))
                    if s >= 4:
                        scalar.wait_ge(out_sems[s % 4], 16 * (s // 4))
                    nc.scalar.copy(
                        out=stage_slice(s),
                        in_=pout_ps[:, (s % 2) * 2 * NK:((s % 2) + 1) * 2 * NK],
                    ).then_inc(stg_sem, 1)

                for s in range(N_SLABS):
                    for t in range(2):
                        g = s * 2 + t
                        scalar.wait_ge(mm_sem, int(cum_mm[g]))
                        if s >= 4 and t == 0:
                            scalar.wait_ge(ds_sem, 2 * (s - 3))
                        nc.scalar.copy(
                            out=slab_sb[t * TRA_BASE:t * TRA_BASE + NORB,
                                        (s % 4) * NK:(s % 4 + 1) * NK],
                            in_=acc_slice(g)).then_inc(cp_sem, 1)
                    if s >= 1:
                        stage(s - 1)
                stage(N_SLABS - 1)

    return nc


def _run(values, cg, sys_idx, i_idx, j_idx, trace=False):
    import ml_dtypes
    bf = ml_dtypes.bfloat16
    va_img, jcol, Ck, spans = _preprocess(values, sys_idx, i_idx, j_idx)
    bd = _make_bd(np.asarray(cg, dtype=np.float32)).astype(bf)
    nchunk = int(Ck.sum()) * CHUNK * NRAD // 128
    nc = _build_program(Ck, nchunk, spans)
    in_maps = [{"va": va_img[c].astype(bf), "jcol": jcol[c], "bd": bd}
               for c in range(N_CORES)]
    res = run_bass_kernel_spmd(nc, in_maps, list(range(N_CORES)), trace=trace)
    outs = [np.asarray(res.results[c]["out"], dtype=np.float32)
            for c in range(N_CORES)]
    return _postprocess(outs), res


def kernel(values, cg, sys_idx, i_idx, j_idx):
    H, _ = _run(np.asarray(values, dtype=np.float32), cg, sys_idx, i_idx, j_idx)
    return H


# revision 17
# speedup vs baseline: 4.1040x; 1.4395x over previous
"""Trainium2 Bass kernel for nn_Blocks2Matrix (scatter_memory).

Strategy (all index math is resolved at trace time, so the scatter becomes a
fully static schedule):
 - Shard systems across the 8 cores (2 systems/core); bucket pair entries by
   (system, row atom). Each pair contributes a direct entry (row=i, col=j)
   and a transposed entry (row=j, col=i, V^T).
 - Device layout ("K-layout"): for each system the [2560, 2560] output is a
   row/col permutation of 25 planes [512, 512] indexed (m1, m2); every entry
   contribution is then matmul-native (8x8 radial tiles).
 - Scatter stage: per slab (sys_local, row atom i), chunks of 16 entries are
   accumulated into PSUM via one-hot matmuls:
       acc[0:40 | 40:80, c] += VA_chunk[128, 80].T @ onehot[128, span]
   VA columns 0:40 hold direct entries' V (radial-major, mu-minor), columns
   40:80 hold transposed entries' V^T; onehot rows are (entry, q) with the
   single 1.0 at col j*8+q. Entries are sorted by col, so non-first chunks
   only touch a narrow column span (first chunk is full-width to zero PSUM).
 - Dense stage: per (slab, pg in {0,1}) one matmul
       pout[(pp, ab) = 100, c] = BD[128, 100].T @ slab_sb[128, 512]
   with BD block-diagonal cg rows 0:40 (direct) and cg^T rows 40:80.
 - fp16 operands (exact for one-hots/iota; ~5e-4 rel on values), fp32 PSUM
   accumulation, fp16 output; host permutes to the reference layout.
"""
import contextlib

import numpy as np

import concourse.bass as bass
import concourse.mybir as mybir
from concourse.bass_utils import run_bass_kernel_spmd

N_SYS, N_ATOMS, NRAD, MU, M1, M2 = 16, 64, 8, 5, 5, 5
S = 32768
NORB = NRAD * M1            # 40
NORB2 = 2 * NORB            # 80 (dir + tra column blocks)
N = N_ATOMS * NORB          # 2560
N_CORES = 8
SYS_PER_CORE = N_SYS // N_CORES
NK = N_ATOMS * NRAD         # 512
CHUNK = 16                  # entries per scatter matmul (K = 128)
N_SLABS = SYS_PER_CORE * N_ATOMS   # 128 per core
KDIM = 128                  # dense-stage contraction rows (padded for FWL)
F32 = mybir.dt.float32
FP16 = mybir.dt.float16


def _preprocess(values, sys_idx, i_idx, j_idx):
    """Build per-core SBUF images.

    Returns (va_img [8,128,nchunk*80] f32, jcol [8,128,nchunk] f32,
             Ck [N_SLABS], spans [nchunk,2]).
    """
    vals = values.reshape(S, MU, NRAD, NRAD)
    sys_idx = np.asarray(sys_idx, dtype=np.int64)
    i_idx = np.asarray(i_idx, dtype=np.int64)
    j_idx = np.asarray(j_idx, dtype=np.int64)

    ent_sys = np.concatenate([sys_idx, sys_idx])
    ent_row = np.concatenate([i_idx, j_idx])
    ent_col = np.concatenate([j_idx, i_idx])
    ent_typ = np.concatenate([np.zeros(S, np.int64), np.ones(S, np.int64)])

    core_of = ent_sys // SYS_PER_CORE
    slab_of = (ent_sys % SYS_PER_CORE) * N_ATOMS + ent_row

    flat = core_of * N_SLABS + slab_of
    counts = np.bincount(flat, minlength=N_CORES * N_SLABS).reshape(N_CORES, N_SLABS)
    Ck = np.maximum(1, -(-counts // CHUNK)).max(axis=0)      # [N_SLABS] joint chunks
    E_slab = Ck * CHUNK
    E_tot = int(E_slab.sum())
    grp_base = np.concatenate([[0], np.cumsum(E_slab)[:-1]])

    order = np.lexsort((ent_col, ent_row, ent_sys))
    oc = core_of[order]
    ogrp = slab_of[order]
    key = oc * N_SLABS + ogrp
    first = np.r_[True, key[1:] != key[:-1]]
    idx = np.arange(key.size)
    start_of_group = np.maximum.accumulate(np.where(first, idx, 0))
    rank = idx - start_of_group
    dest = grp_base[ogrp] + rank

    # VA[e, q, 0:40]  = V[mu, p, q]   (p*5+mu, direct entries)
    # VA[e, q, 40:80] = V[mu, q', p'] (transposed entries)
    VA_dir = vals.transpose(0, 3, 2, 1).reshape(S, NRAD, NORB)
    VA_tra = vals.transpose(0, 2, 3, 1).reshape(S, NRAD, NORB)
    ent_VA = np.zeros((2 * S, NRAD, NORB2), dtype=np.float32)
    ent_VA[:S, :, :NORB] = VA_dir
    ent_VA[S:, :, NORB:] = VA_tra

    va = np.zeros((N_CORES, E_tot, NRAD, NORB2), dtype=np.float32)
    va[oc, dest] = ent_VA[order]
    jq = np.zeros((N_CORES, E_tot, NRAD), dtype=np.float32)
    jq[oc, dest] = (ent_col[order, None] * NRAD + np.arange(NRAD)[None, :]).astype(np.float32)

    va_flat = va.reshape(N_CORES, E_tot * NRAD, NORB2)
    nchunk = E_tot * NRAD // 128
    va_img = va_flat.reshape(N_CORES, nchunk, 128, NORB2).transpose(0, 2, 1, 3) \
                    .reshape(N_CORES, 128, nchunk * NORB2).copy()
    jcol = jq.reshape(N_CORES, nchunk, 128).transpose(0, 2, 1).copy()

    # per-chunk one-hot column spans (union over cores; entries sorted by col
    # within each slab). Padding entries (jc=0) never match sliced chunks.
    chunk_of = (dest // CHUNK).astype(np.int64)
    cmin = np.full(nchunk, NK, np.int64)
    cmax = np.full(nchunk, -1, np.int64)
    ecol = ent_col[order] * NRAD
    np.minimum.at(cmin, chunk_of, ecol)
    np.maximum.at(cmax, chunk_of, ecol + NRAD)
    empty = cmax < 0
    cmin[empty], cmax[empty] = 0, NRAD
    spans = np.stack([(cmin // 4) * 4, np.minimum(NK, -(-cmax // 4) * 4)], axis=1)
    return va_img, jcol, Ck, spans


def _make_bd(cg):
    """BD [128, 200]: rows 0:40 direct cg, rows 40:80 transposed cg, rows
    80:128 zero (pads K to 128 for fast weight load); cols pg*100."""
    cgm_dir = cg.reshape(M1 * M2, MU).T.astype(np.float32)                    # [mu, a*5+b]
    cgm_tra = np.ascontiguousarray(cg.transpose(1, 0, 2)).reshape(M1 * M2, MU).T.astype(np.float32)
    bd = np.zeros((KDIM, 2 * 100), dtype=np.float32)
    for pg in range(2):
        for pp in range(4):
            p = pg * 4 + pp
            bd[p * 5:(p + 1) * 5, pg * 100 + pp * 25: pg * 100 + (pp + 1) * 25] = cgm_dir
            bd[NORB + p * 5:NORB + (p + 1) * 5, pg * 100 + pp * 25: pg * 100 + (pp + 1) * 25] = cgm_tra
    return bd


def _postprocess(outs):
    """outs: [8][N_SLABS*2*100, NK] f32 -> H [N_SYS, N, N]."""
    K = np.stack(outs).reshape(N_CORES * SYS_PER_CORE, N_ATOMS, 2, 4, M1, M2, N_ATOMS, NRAD)
    return np.ascontiguousarray(
        K.transpose(0, 1, 2, 3, 4, 6, 7, 5)
    ).reshape(N_SYS, N, N)


def _build_program(Ck, nchunk, spans):
    """Raw-bass SPMD program (explicit semaphores).

    Pipeline per chunk k / slab s:
      DVE : one-hot[k%8] = is_equal(iota, jc[:, k])            -> oh_sem
      PE  : acc[s%4] += va[k].T @ oh[k%8]   (PSUM accumulate)  -> mm_sem
      ACT/DVE (alternating s): slab_sb[s%4][0:80] <- acc copy  -> cpA/cpD_sem
      PE  : pout[s%2][pg] = BD[pg].T @ slab_sb[s%4]            -> ds_sem
      ACT : stage[s%4] <- pout[s%2]  (fp32 -> fp16)            -> stg_sem
      SYNC: DRAM out rows [s*200, (s+1)*200) <- stage[s%4]     -> out_sems[s%4]
    """
    nc = bass.Bass()
    W = nchunk * NORB2

    va_d = nc.declare_dram_parameter("va", [128, W], FP16, isOutput=False)
    jc_d = nc.declare_dram_parameter("jcol", [128, nchunk], F32, isOutput=False)
    bd_d = nc.declare_dram_parameter("bd", [KDIM, 200], FP16, isOutput=False)
    out_d = nc.declare_dram_parameter("out", [N_SLABS * 2 * 100, NK], FP16, isOutput=True)

    # static schedule
    first_k = np.concatenate([[0], np.cumsum(Ck)[:-1]]).astype(int)
    cum_mm = np.cumsum(Ck).astype(int)               # mm_sem value after slab s
    n_chunks = int(cum_mm[-1])
    kspan = spans.copy()
    for s in range(N_SLABS):
        kspan[first_k[s]] = (0, NK)

    # slab-copy engine assignment + per-engine 1-based completion index
    cp_eng = ['A' if s % 2 == 0 else 'D' for s in range(N_SLABS)]
    cp_idx = np.zeros(N_SLABS, dtype=int)
    ca = cd = 0
    for s in range(N_SLABS):
        if cp_eng[s] == 'A':
            ca += 1; cp_idx[s] = ca
        else:
            cd += 1; cp_idx[s] = cd

    # va is loaded in 16 column-stripes with individual semaphores
    n_va_tiles = 16
    va_bnd = [W * t // n_va_tiles for t in range(n_va_tiles + 1)]
    va_bnd = [b - b % NORB2 for b in va_bnd[:-1]] + [W]   # chunk-aligned

    with (
        nc.sbuf_tensor([128, W], FP16) as va_sb,
        nc.sbuf_tensor([128, nchunk], F32) as jc_sb,
        nc.sbuf_tensor([KDIM, 200], FP16) as bd_sb,
        nc.sbuf_tensor([128, NK], FP16) as iota_sb,
        nc.sbuf_tensor([128, 8 * NK], FP16) as oh_sb,
        nc.sbuf_tensor([KDIM, 4 * NK], FP16) as slab_sb,
        nc.sbuf_tensor([100, 4 * 2 * NK], FP16) as stage_sb,
        nc.psum_tensor([NORB2, 4 * NK], F32) as acc_ps,
        nc.psum_tensor([100, 2 * 2 * NK], F32) as pout_ps,
        nc.semaphore("cst_sem") as cst_sem,
        nc.semaphore("oh_sem") as oh_sem,
        nc.semaphore("mm_sem") as mm_sem,
        nc.semaphore("ds_sem") as ds_sem,
        nc.semaphore("cpA_sem") as cpA_sem,
        nc.semaphore("cpD_sem") as cpD_sem,
        nc.semaphore("stg_sem") as stg_sem,
        nc.Block() as block,
    ):
        with contextlib.ExitStack() as stk:
            va_sems = [stk.enter_context(nc.semaphore(f"va_sem{t}"))
                       for t in range(n_va_tiles)]
            out_sems = [stk.enter_context(nc.semaphore(f"out_sem{i}"))
                        for i in range(4)]
            cp_sems = {'A': cpA_sem, 'D': cpD_sem}

            def oh_slice(k, c0, c1):
                base = (k % 8) * NK
                return oh_sb[:, base + c0:base + c1]

            def cp_wait(engine, s):
                engine.wait_ge(cp_sems[cp_eng[s]], int(cp_idx[s]))

            @block.gpsimd
            def _(gpsimd):
                nc.gpsimd.iota(iota_sb[:], pattern=[[1, NK]], base=0,
                               channel_multiplier=0,
                               allow_small_or_imprecise_dtypes=True).then_inc(cst_sem, 1)

            @block.sync
            def _(sync):
                for t in range(n_va_tiles):
                    sync.dma_start(out=va_sb[:, va_bnd[t]:va_bnd[t + 1]],
                                   in_=va_d[:, va_bnd[t]:va_bnd[t + 1]]).then_inc(va_sems[t], 16)
                sync.dma_start(out=jc_sb[:], in_=jc_d[:]).then_inc(cst_sem, 16)
                sync.dma_start(out=bd_sb[:], in_=bd_d[:]).then_inc(cst_sem, 16)
                for s in range(N_SLABS):
                    sync.wait_ge(stg_sem, s + 1)
                    dst = out_d[s * 200:(s + 1) * 200, :] \
                        .rearrange("(pg r) c -> r pg c", pg=2)
                    sync.dma_start(
                        out=dst,
                        in_=stage_sb[:, (s % 4) * 2 * NK:((s % 4) + 1) * 2 * NK]
                            .rearrange("r (pg c) -> r pg c", pg=2),
                    ).then_inc(out_sems[s % 4], 16)

            @block.vector
            def _(vector):
                vector.wait_ge(cst_sem, 33)

                def dve_copy(s):
                    vector.wait_ge(mm_sem, int(cum_mm[s]))
                    if s >= 4:
                        vector.wait_ge(ds_sem, 2 * (s - 3))
                    nc.vector.tensor_copy(
                        out=slab_sb[0:NORB2, (s % 4) * NK:(s % 4 + 1) * NK],
                        in_=acc_ps[:, (s % 4) * NK:(s % 4 + 1) * NK],
                    ).then_inc(cpD_sem, 1)

                k = 0
                for s in range(N_SLABS):
                    for kk in range(int(Ck[s])):
                        c0, c1 = int(kspan[k][0]), int(kspan[k][1])
                        if k >= 8:
                            vector.wait_ge(mm_sem, k - 7)
                        nc.vector.tensor_scalar(
                            out=oh_slice(k, c0, c1), in0=iota_sb[:, c0:c1],
                            scalar1=jc_sb[:, k:k + 1], scalar2=None,
                            op0=mybir.AluOpType.is_equal).then_inc(oh_sem, 1)
                        k += 1
                    if s >= 1 and cp_eng[s - 1] == 'D':
                        dve_copy(s - 1)
                if cp_eng[N_SLABS - 1] == 'D':
                    dve_copy(N_SLABS - 1)

            @block.tensor
            def _(tensor):
                tensor.wait_ge(cst_sem, 33)

                def dense(s):
                    cp_wait(tensor, s)
                    if s >= 2:
                        tensor.wait_ge(stg_sem, s - 1)
                    for pg in range(2):
                        nc.tensor.matmul(
                            pout_ps[:, ((s % 2) * 2 + pg) * NK:((s % 2) * 2 + pg + 1) * NK],
                            bd_sb[:, pg * 100:(pg + 1) * 100],
                            slab_sb[:, (s % 4) * NK:(s % 4 + 1) * NK],
                            start=True, stop=True).then_inc(ds_sem, 1)

                cur_tile = -1
                k = 0
                for s in range(N_SLABS):
                    nck = int(Ck[s])
                    for kk in range(nck):
                        tt = 0
                        while va_bnd[tt + 1] <= k * NORB2:
                            tt += 1
                        if tt != cur_tile:
                            tensor.wait_ge(va_sems[tt], 16)
                            cur_tile = tt
                        tensor.wait_ge(oh_sem, k + 1)
                        if kk == 0 and s >= 4:
                            cp_wait(tensor, s - 4)
                        c0, c1 = int(kspan[k][0]), int(kspan[k][1])
                        nc.tensor.matmul(
                            acc_ps[:, (s % 4) * NK + c0:(s % 4) * NK + c1],
                            va_sb[:, k * NORB2:(k + 1) * NORB2],
                            oh_slice(k, c0, c1),
                            start=(kk == 0), stop=(kk == nck - 1),
                            skip_group_check=True).then_inc(mm_sem, 1)
                        k += 1
                    if s >= 1:
                        dense(s - 1)
                dense(N_SLABS - 1)

            @block.scalar
            def _(scalar):
                scalar.wait_ge(cst_sem, 33)
                # zero rows 64:128 of each slab slot once; copies rewrite
                # 64:80 every slab, rows 80:128 stay zero (K padding)
                for sl in range(4):
                    nc.scalar.activation(
                        out=slab_sb[64:128, sl * NK:(sl + 1) * NK],
                        in_=iota_sb[64:128, :],
                        func=mybir.ActivationFunctionType.Copy, scale=0.0)

                def act_copy(s):
                    scalar.wait_ge(mm_sem, int(cum_mm[s]))
                    if s >= 4:
                        scalar.wait_ge(ds_sem, 2 * (s - 3))
                    nc.scalar.copy(
                        out=slab_sb[0:NORB2, (s % 4) * NK:(s % 4 + 1) * NK],
                        in_=acc_ps[:, (s % 4) * NK:(s % 4 + 1) * NK],
                    ).then_inc(cpA_sem, 1)

                def stage(s):
                    scalar.wait_ge(ds_sem, 2 * (s + 1))
                    if s >= 4:
                        scalar.wait_ge(out_sems[s % 4], 16 * (s // 4))
                    nc.scalar.copy(
                        out=stage_sb[:, (s % 4) * 2 * NK:((s % 4) + 1) * 2 * NK],
                        in_=pout_ps[:, (s % 2) * 2 * NK:((s % 2) + 1) * 2 * NK],
                    ).then_inc(stg_sem, 1)

                for s in range(N_SLABS):
                    if cp_eng[s] == 'A':
                        act_copy(s)
                    if s >= 1:
                        stage(s - 1)
                stage(N_SLABS - 1)

    return nc


def _run(values, cg, sys_idx, i_idx, j_idx, trace=False):
    import ml_dtypes
    f16 = np.float16
    va_img, jcol, Ck, spans = _preprocess(values, sys_idx, i_idx, j_idx)
    bd = _make_bd(np.asarray(cg, dtype=np.float32)).astype(f16)
    nchunk = int(Ck.sum()) * CHUNK * NRAD // 128
    nc = _build_program(Ck, nchunk, spans)
    in_maps = [{"va": va_img[c].astype(f16), "jcol": jcol[c], "bd": bd}
               for c in range(N_CORES)]
    res = run_bass_kernel_spmd(nc, in_maps, list(range(N_CORES)), trace=trace)
    outs = [np.asarray(res.results[c]["out"], dtype=np.float32)
            for c in range(N_CORES)]
    return _postprocess(outs), res


def kernel(values, cg, sys_idx, i_idx, j_idx):
    H, _ = _run(np.asarray(values, dtype=np.float32), cg, sys_idx, i_idx, j_idx)
    return H
